# revision 12
# baseline (speedup 1.0000x reference)
"""FP6Linear (fake-quant-dequant weight + linear) on 8 Trainium2 NeuronCores.

Strategy: column-parallel tensor parallelism. Each core gets a 2048-row shard
of W (out_features) and bias, with x replicated. x is pre-cast to bf16 and
pre-tiled on host into [MT, 128, KB*128] so each m-tile's stationary operand
loads as one fully contiguous 1 MiB DMA.

The FP6 fake-quant-dequant runs on device. The per-tensor scale needs the
abs-max over ALL of W; the sharding replicates one row of W — the row holding
the global abs-max — to every core, so max(|w_extra|) is exactly the global
abs-max with no cross-core traffic (a collective was measured to trip the
board-level GPIO power throttle) and no pass over the shard.

Dequant is exact vs the jax reference but clip-free: scale = amax/16 means
|W*inv| <= 16(1+ulp), and the rne magic-number path maps the +-ulp overhangs
to the same q as the clipped path (16+eps -> q=63, -eps -> q=-0 -> w=c).
Chain: t = W*inv + 16 -> y = t*63/32 + 2^23 (rne) -> q = y - 2^23 (exact
bf16 ints) -> w = q*a + c, balanced across Vector and Scalar (GpSimd
tensor_scalar measured 25x slower).

The matmul runs as two passes over x, one per N-half (1024 columns). In
pass 1 the dequant of half 0 streams concurrently: PSUM groups are
[128, 1024] (2 banks), so 4 m-tiles are in flight and the PE consumes each
arriving half-k-block at 8 matmuls (~1.73 us) while a half-width dequant
chunk takes ~1.5 us to produce — after a ~13 us ramp the PE never starves.
Half 1 dequantizes in the background of pass 1; pass 2 then runs at full
rate. Steady-state issue is ~216 ns per N=512 matmul (~99% of the bf16
streaming peak). DMA rings are split (sync: W/wx/y; scalar: bias/x;
gpsimd: only the partition all-reduce — its SWDGE descriptor-gen is far
too slow for bulk tiles) so the latency-critical weight path never
queues behind bulk traffic.
"""

import numpy as np
import ml_dtypes

import concourse.bacc as bacc
import concourse.bass as bass
import concourse.bass_isa as bass_isa
import concourse.mybir as mybir
import concourse.tile as tile
from concourse import bass_utils

# Problem shapes (hardcoded per contract)
B, S, D_IN, D_OUT = 4, 2048, 4096, 16384
M = B * S               # 8192 rows of x
K = D_IN                # 4096 contraction
N_CORES = 8
N = D_OUT // N_CORES    # 2048 out-features per core
P = 128
KB = K // P             # 32 k-blocks
MT = M // P             # 64 m-tiles
NH = N // 2             # 1024 out-features per pass
NQS = 512               # matmul free dim (one PSUM bank)
WIN = 4                 # m-tiles interleaved during the dequant window

FP32 = mybir.dt.float32
BF16 = mybir.dt.bfloat16
BF16_NP = ml_dtypes.bfloat16
MAGIC = 8388608.0       # 2^23: fp32 add rounds the sum to integer (rne)

_COMPILED = {}


def _build():
    nc = bacc.Bacc(
        "TRN2",
        target_bir_lowering=False,
        debug=False,
        enable_asserts=False,
        num_devices=N_CORES,
    )
    xt_d = nc.dram_tensor("xt", [MT, P, KB * P], BF16, kind="ExternalInput").ap()
    wT_d = nc.dram_tensor("wT", [K, N], FP32, kind="ExternalInput").ap()
    wx_d = nc.dram_tensor("wx", [1, K], FP32, kind="ExternalInput").ap()
    bias_d = nc.dram_tensor("bias", [1, N], FP32, kind="ExternalInput").ap()
    y_d = nc.dram_tensor("y", [M, N], FP32, kind="ExternalOutput").ap()

    with tile.TileContext(nc) as tc:
        with (
            tc.tile_pool(name="const", bufs=1) as const,
            tc.tile_pool(name="wt", bufs=1) as wt_pool,
            tc.tile_pool(name="wl", bufs=4) as wl_pool,
            tc.tile_pool(name="xt", bufs=5) as xt_pool,
            tc.tile_pool(name="ot", bufs=2) as ot_pool,
            tc.tile_pool(name="psum", bufs=4, space="PSUM") as psum,
        ):
            # ---- global abs-max from the replicated argmax row alone ----
            # (first on the sync DMA ring: the scale chain gates everything)
            wx_sb = const.tile([P, K // P], FP32)
            nc.sync.dma_start(wx_sb[:], wx_d.rearrange("a (p b) -> p (a b)", p=P))
            wx_red = const.tile([P, 1], FP32)
            nc.vector.tensor_reduce(
                wx_red[:], wx_sb[:], mybir.AxisListType.X,
                mybir.AluOpType.max, apply_absolute_value=True,
            )
            g_amax = const.tile([P, 1], FP32)
            nc.gpsimd.partition_all_reduce(
                g_amax[:], wx_red[:], channels=P, reduce_op=bass_isa.ReduceOp.max
            )

            # ---- scale = where(amax > 0, amax/16, 1); inv = 1/scale ----
            # amax = |global max| > 0 for any nonzero W (randn inputs), so
            # the where() is identically amax/16; amax*(1/16) == amax/16
            # exactly (power-of-two divisor).
            scale_t = const.tile([P, 1], FP32)
            nc.vector.tensor_scalar(
                scale_t[:], g_amax[:], 1.0 / 16.0, None, mybir.AluOpType.mult
            )
            inv_t = const.tile([P, 1], FP32)
            nc.vector.reciprocal(inv_t[:], scale_t[:])
            a_t = const.tile([P, 1], FP32)
            nc.vector.tensor_scalar(a_t[:], scale_t[:], 32.0 / 63.0, None, mybir.AluOpType.mult)
            c_t = const.tile([P, 1], FP32)
            nc.vector.tensor_scalar(c_t[:], scale_t[:], -16.0, None, mybir.AluOpType.mult)

            # bias on the scalar DMA ring (off the W path)
            bias_rep = const.tile([P, N], FP32)
            nc.scalar.dma_start(bias_rep[:], bias_d.to_broadcast((P, N)))

            # ---- dequantize into bf16 W.T SBUF cache, in (half, kb) chunks ----
            # t = W*inv + 16; y = t*63/32 + 2^23 (rne); q = y - 2^23;
            # w = q*a + c   with a = 32/63*scale, c = -16*scale
            wt_sb = wt_pool.tile([P, KB, N], BF16)

            def dequant(kb, lo):
                hi = lo + NH
                wl = wl_pool.tile([P, NH], FP32, tag="wl")
                nc.sync.dma_start(wl[:], wT_d[kb * P : (kb + 1) * P, lo:hi])
                nc.vector.tensor_scalar(
                    wl[:], wl[:], inv_t[:], 16.0,
                    mybir.AluOpType.mult, mybir.AluOpType.add,
                )
                nc.scalar.activation(
                    wl[:], wl[:], mybir.ActivationFunctionType.Copy,
                    scale=63.0 / 32.0, bias=MAGIC,
                )
                nc.vector.tensor_scalar(
                    wt_sb[:, kb, lo:hi], wl[:], -MAGIC, None, mybir.AluOpType.add
                )
                # final affine split across vector + scalar to balance engines
                sp = lo + 512
                nc.vector.tensor_scalar(
                    wt_sb[:, kb, lo:sp], wt_sb[:, kb, lo:sp], a_t[:], c_t[:],
                    mybir.AluOpType.mult, mybir.AluOpType.add,
                )
                nc.scalar.activation(
                    wt_sb[:, kb, sp:hi], wt_sb[:, kb, sp:hi],
                    mybir.ActivationFunctionType.Identity,
                    scale=a_t[:], bias=c_t[:],
                )

            def load_xt(mi):
                xt_t = xt_pool.tile([P, KB * P], BF16, tag="xt")
                nc.scalar.dma_start(xt_t[:], xt_d[mi])
                return xt_t

            def mm(ps, xt_t, kb, q, lo):
                nc.tensor.matmul(
                    ps[:, q * NQS : (q + 1) * NQS],
                    xt_t[:, kb * P : (kb + 1) * P],
                    wt_sb[:, kb, lo + q * NQS : lo + (q + 1) * NQS],
                    start=(kb == 0),
                    stop=(kb == KB - 1),
                )

            def evict(mi, ps, lo):
                ot = ot_pool.tile([P, NH], FP32, tag="ot")
                nc.vector.tensor_tensor(
                    ot[:], ps[:], bias_rep[:, lo : lo + NH], mybir.AluOpType.add
                )
                nc.sync.dma_start(y_d[mi * P : (mi + 1) * P, lo : lo + NH], ot[:])

            # ---- pass 1 (N-half 0): dequant streams concurrently ----
            for kb in range(KB):
                dequant(kb, 0)

            # window: WIN m-tiles interleaved kb-major so the PE consumes
            # each arriving half-k-block at the PSUM-capacity limit
            xts = [load_xt(i) for i in range(WIN)]
            pss = [
                psum.tile([P, NH], FP32, tag="ps", name=f"psw{i}")
                for i in range(WIN)
            ]
            for kb in range(KB):
                for i in range(WIN):
                    mm(pss[i], xts[i], kb, 0, 0)
                    mm(pss[i], xts[i], kb, 1, 0)
            for i in range(WIN):
                evict(i, pss[i], 0)

            for mi in range(WIN, MT):
                xt_t = load_xt(mi)
                ps = psum.tile([P, NH], FP32, tag="ps")
                for kb in range(KB):
                    mm(ps, xt_t, kb, 0, 0)
                    mm(ps, xt_t, kb, 1, 0)
                # half-1 dequant chunks interleaved into pass 1's engine
                # queues so they overlap the matmul stream (emitted before
                # the eviction so y-writes never block W loads on the ring)
                if WIN <= mi < WIN + KB:
                    dequant(mi - WIN, NH)
                evict(mi, ps, 0)

            # ---- pass 2 (N-half 1): all weights resident, full rate ----
            for mi in range(MT):
                xt_t = load_xt(mi)
                ps = psum.tile([P, NH], FP32, tag="ps")
                for kb in range(KB):
                    mm(ps, xt_t, kb, 0, NH)
                    mm(ps, xt_t, kb, 1, NH)
                evict(mi, ps, NH)

    nc.compile()
    return nc


def _get_compiled():
    if "nc" not in _COMPILED:
        _COMPILED["nc"] = _build()
    return _COMPILED["nc"]


def _make_in_maps(x, W, bias):
    xb = x.reshape(M, K).astype(BF16_NP)
    # [mi, pm, kb, pk] -> [mi, pk, kb, pm]: per-m-tile contiguous K-major tiles
    xt = np.ascontiguousarray(
        xb.reshape(MT, P, KB, P).transpose(0, 3, 2, 1)
    ).reshape(MT, P, KB * P)
    W = np.ascontiguousarray(W.astype(np.float32, copy=False))
    # replicate the W row holding the global abs-max so every core can form
    # the exact global max from local data
    gmax_row = int(np.argmax(np.abs(W)) // K)
    wx = np.ascontiguousarray(W[gmax_row : gmax_row + 1, :])
    in_maps = []
    for c in range(N_CORES):
        wT = np.ascontiguousarray(W[c * N : (c + 1) * N, :].T)
        b = np.ascontiguousarray(bias[c * N : (c + 1) * N].astype(np.float32, copy=False)).reshape(1, N)
        in_maps.append({"xt": xt, "wT": wT, "wx": wx, "bias": b})
    return in_maps


def kernel(x: np.ndarray, W: np.ndarray, bias: np.ndarray) -> np.ndarray:
    assert x.shape == (B, S, D_IN) and W.shape == (D_OUT, D_IN) and bias.shape == (D_OUT,)
    nc = _get_compiled()
    in_maps = _make_in_maps(x, W, bias)
    res = bass_utils.run_bass_kernel_spmd(nc, in_maps, core_ids=list(range(N_CORES)))
    y = np.concatenate([res.results[c]["y"] for c in range(N_CORES)], axis=1)
    return y.reshape(B, S, D_OUT)


# revision 13
# speedup vs baseline: 1.0156x; 1.0156x over previous
"""FP6Linear (fake-quant-dequant weight + linear) on 8 Trainium2 NeuronCores.

Strategy: column-parallel tensor parallelism. Each core gets a 2048-row shard
of W (out_features) and bias, with x replicated. x is pre-cast to bf16 and
pre-tiled on host into [MT, 128, KB*128] so each m-tile's stationary operand
loads as one fully contiguous 1 MiB DMA.

The FP6 fake-quant-dequant runs on device. The per-tensor scale needs the
abs-max over ALL of W; the sharding replicates one row of W — the row holding
the global abs-max — to every core, so max(|w_extra|) is exactly the global
abs-max with no cross-core traffic (a collective was measured to trip the
board-level GPIO power throttle) and no pass over the shard.

Dequant is exact vs the jax reference but clip-free: scale = amax/16 means
|W*inv| <= 16(1+ulp), and the rne magic-number path maps the +-ulp overhangs
to the same q as the clipped path (16+eps -> q=63, -eps -> q=-0 -> w=c).
Chain: t = W*inv + 16 -> y = t*63/32 + 2^23 (rne) -> q = y - 2^23 (exact
bf16 ints) -> w = q*a + c, balanced across Vector and Scalar (GpSimd
tensor_scalar measured 25x slower).

The matmul runs as two passes over x, one per N-half (1024 columns). In
pass 1 the dequant of half 0 streams concurrently: PSUM groups are
[128, 1024] (2 banks), so 4 m-tiles are in flight and the PE consumes each
arriving half-k-block at 8 matmuls (~1.73 us) while a half-width dequant
chunk takes ~1.5 us to produce — after a ~13 us ramp the PE never starves.
Half 1 dequantizes in the background of pass 1; pass 2 then runs at full
rate. Steady-state issue is ~216 ns per N=512 matmul (~99% of the bf16
streaming peak). DMA rings are split (sync: W/wx/y; scalar: bias/x;
gpsimd: only the partition all-reduce — its SWDGE descriptor-gen is far
too slow for bulk tiles) so the latency-critical weight path never
queues behind bulk traffic.
"""

import numpy as np
import ml_dtypes

import concourse.bacc as bacc
import concourse.bass as bass
import concourse.bass_isa as bass_isa
import concourse.mybir as mybir
import concourse.tile as tile
from concourse import bass_utils

# Problem shapes (hardcoded per contract)
B, S, D_IN, D_OUT = 4, 2048, 4096, 16384
M = B * S               # 8192 rows of x
K = D_IN                # 4096 contraction
N_CORES = 8
N = D_OUT // N_CORES    # 2048 out-features per core
P = 128
KB = K // P             # 32 k-blocks
MT = M // P             # 64 m-tiles
NH = N // 2             # 1024 out-features per pass
NQS = 512               # matmul free dim (one PSUM bank)
WIN = 4                 # m-tiles interleaved during the dequant window

FP32 = mybir.dt.float32
BF16 = mybir.dt.bfloat16
BF16_NP = ml_dtypes.bfloat16
MAGIC = 8388608.0       # 2^23: fp32 add rounds the sum to integer (rne)

_COMPILED = {}


def _build():
    nc = bacc.Bacc(
        "TRN2",
        target_bir_lowering=False,
        debug=False,
        enable_asserts=False,
        num_devices=N_CORES,
    )
    xt_d = nc.dram_tensor("xt", [MT, P, KB * P], BF16, kind="ExternalInput").ap()
    wT_d = nc.dram_tensor("wT", [K, N], FP32, kind="ExternalInput").ap()
    wx_d = nc.dram_tensor("wx", [1, K], FP32, kind="ExternalInput").ap()
    bias_d = nc.dram_tensor("bias", [1, N], FP32, kind="ExternalInput").ap()
    y_d = nc.dram_tensor("y", [M, N], FP32, kind="ExternalOutput").ap()

    with tile.TileContext(nc) as tc:
        with (
            tc.tile_pool(name="const", bufs=1) as const,
            tc.tile_pool(name="wt", bufs=1) as wt_pool,
            tc.tile_pool(name="wl", bufs=3) as wl_pool,
            tc.tile_pool(name="tmp", bufs=2) as tmp_pool,
            tc.tile_pool(name="xt", bufs=5) as xt_pool,
            tc.tile_pool(name="ot", bufs=2) as ot_pool,
            tc.tile_pool(name="psum", bufs=4, space="PSUM") as psum,
        ):
            # ---- global abs-max from the replicated argmax row alone ----
            # (first on the sync DMA ring: the scale chain gates everything)
            wx_sb = const.tile([P, K // P], FP32)
            nc.sync.dma_start(wx_sb[:], wx_d.rearrange("a (p b) -> p (a b)", p=P))
            wx_red = const.tile([P, 1], FP32)
            nc.vector.tensor_reduce(
                wx_red[:], wx_sb[:], mybir.AxisListType.X,
                mybir.AluOpType.max, apply_absolute_value=True,
            )
            g_amax = const.tile([P, 1], FP32)
            nc.gpsimd.partition_all_reduce(
                g_amax[:], wx_red[:], channels=P, reduce_op=bass_isa.ReduceOp.max
            )

            # ---- scale = where(amax > 0, amax/16, 1); inv = 1/scale ----
            # amax = |global max| > 0 for any nonzero W (randn inputs), so
            # the where() is identically amax/16; amax*(1/16) == amax/16
            # exactly (power-of-two divisor).
            scale_t = const.tile([P, 1], FP32)
            nc.vector.tensor_scalar(
                scale_t[:], g_amax[:], 1.0 / 16.0, None, mybir.AluOpType.mult
            )
            inv_t = const.tile([P, 1], FP32)
            nc.vector.reciprocal(inv_t[:], scale_t[:])
            a_t = const.tile([P, 1], FP32)
            nc.vector.tensor_scalar(a_t[:], scale_t[:], 32.0 / 63.0, None, mybir.AluOpType.mult)
            c_t = const.tile([P, 1], FP32)
            nc.vector.tensor_scalar(c_t[:], scale_t[:], -16.0, None, mybir.AluOpType.mult)

            # bias on the scalar DMA ring (off the W path)
            bias_rep = const.tile([P, N], FP32)
            nc.scalar.dma_start(bias_rep[:], bias_d.to_broadcast((P, N)))

            # ---- dequantize into bf16 W.T SBUF cache, in (half, kb) chunks ----
            # t = W*inv + 16; y = t*63/32 + 2^23 (rne); q = y - 2^23;
            # w = q*a + c   with a = 32/63*scale, c = -16*scale
            wt_sb = wt_pool.tile([P, KB, N], BF16)

            def dequant(kb, lo):
                hi = lo + NH
                wl = wl_pool.tile([P, NH], FP32, tag="wl")
                nc.sync.dma_start(wl[:], wT_d[kb * P : (kb + 1) * P, lo:hi])
                t = tmp_pool.tile([P, NH], FP32, tag="t")
                nc.vector.tensor_scalar(
                    t[:], wl[:], inv_t[:], 16.0,
                    mybir.AluOpType.mult, mybir.AluOpType.add,
                )
                nc.scalar.activation(
                    t[:], t[:], mybir.ActivationFunctionType.Copy,
                    scale=63.0 / 32.0, bias=MAGIC,
                )
                nc.vector.tensor_scalar(
                    wt_sb[:, kb, lo:hi], t[:], -MAGIC, None, mybir.AluOpType.add
                )
                # final affine split across vector + scalar to balance engines
                sp = lo + 512
                nc.vector.tensor_scalar(
                    wt_sb[:, kb, lo:sp], wt_sb[:, kb, lo:sp], a_t[:], c_t[:],
                    mybir.AluOpType.mult, mybir.AluOpType.add,
                )
                nc.scalar.activation(
                    wt_sb[:, kb, sp:hi], wt_sb[:, kb, sp:hi],
                    mybir.ActivationFunctionType.Identity,
                    scale=a_t[:], bias=c_t[:],
                )

            def load_xt(mi):
                xt_t = xt_pool.tile([P, KB * P], BF16, tag="xt")
                nc.scalar.dma_start(xt_t[:], xt_d[mi])
                return xt_t

            def mm(ps, xt_t, kb, q, lo):
                nc.tensor.matmul(
                    ps[:, q * NQS : (q + 1) * NQS],
                    xt_t[:, kb * P : (kb + 1) * P],
                    wt_sb[:, kb, lo + q * NQS : lo + (q + 1) * NQS],
                    start=(kb == 0),
                    stop=(kb == KB - 1),
                )

            def evict(mi, ps, lo):
                ot = ot_pool.tile([P, NH], FP32, tag="ot")
                nc.vector.tensor_tensor(
                    ot[:], ps[:], bias_rep[:, lo : lo + NH], mybir.AluOpType.add
                )
                nc.sync.dma_start(y_d[mi * P : (mi + 1) * P, lo : lo + NH], ot[:])

            # ---- pass 1 (N-half 0): dequant streams concurrently ----
            for kb in range(KB):
                dequant(kb, 0)

            # window: WIN m-tiles interleaved kb-major so the PE consumes
            # each arriving half-k-block at the PSUM-capacity limit
            xts = [load_xt(i) for i in range(WIN)]
            pss = [
                psum.tile([P, NH], FP32, tag="ps", name=f"psw{i}")
                for i in range(WIN)
            ]
            for kb in range(KB):
                for i in range(WIN):
                    mm(pss[i], xts[i], kb, 0, 0)
                    mm(pss[i], xts[i], kb, 1, 0)
            for i in range(WIN):
                evict(i, pss[i], 0)

            for mi in range(WIN, MT):
                xt_t = load_xt(mi)
                ps = psum.tile([P, NH], FP32, tag="ps")
                for kb in range(KB):
                    mm(ps, xt_t, kb, 0, 0)
                    mm(ps, xt_t, kb, 1, 0)
                # half-1 dequant chunks interleaved into pass 1's engine
                # queues so they overlap the matmul stream (emitted before
                # the eviction so y-writes never block W loads on the ring)
                if WIN <= mi < WIN + KB:
                    dequant(mi - WIN, NH)
                evict(mi, ps, 0)

            # ---- pass 2 (N-half 1): all weights resident, full rate ----
            for mi in range(MT):
                xt_t = load_xt(mi)
                ps = psum.tile([P, NH], FP32, tag="ps")
                for kb in range(KB):
                    mm(ps, xt_t, kb, 0, NH)
                    mm(ps, xt_t, kb, 1, NH)
                evict(mi, ps, NH)

    nc.compile()
    return nc


def _get_compiled():
    if "nc" not in _COMPILED:
        _COMPILED["nc"] = _build()
    return _COMPILED["nc"]


def _make_in_maps(x, W, bias):
    xb = x.reshape(M, K).astype(BF16_NP)
    # [mi, pm, kb, pk] -> [mi, pk, kb, pm]: per-m-tile contiguous K-major tiles
    xt = np.ascontiguousarray(
        xb.reshape(MT, P, KB, P).transpose(0, 3, 2, 1)
    ).reshape(MT, P, KB * P)
    W = np.ascontiguousarray(W.astype(np.float32, copy=False))
    # replicate the W row holding the global abs-max so every core can form
    # the exact global max from local data
    gmax_row = int(np.argmax(np.abs(W)) // K)
    wx = np.ascontiguousarray(W[gmax_row : gmax_row + 1, :])
    in_maps = []
    for c in range(N_CORES):
        wT = np.ascontiguousarray(W[c * N : (c + 1) * N, :].T)
        b = np.ascontiguousarray(bias[c * N : (c + 1) * N].astype(np.float32, copy=False)).reshape(1, N)
        in_maps.append({"xt": xt, "wT": wT, "wx": wx, "bias": b})
    return in_maps


def kernel(x: np.ndarray, W: np.ndarray, bias: np.ndarray) -> np.ndarray:
    assert x.shape == (B, S, D_IN) and W.shape == (D_OUT, D_IN) and bias.shape == (D_OUT,)
    nc = _get_compiled()
    in_maps = _make_in_maps(x, W, bias)
    res = bass_utils.run_bass_kernel_spmd(nc, in_maps, core_ids=list(range(N_CORES)))
    y = np.concatenate([res.results[c]["y"] for c in range(N_CORES)], axis=1)
    return y.reshape(B, S, D_OUT)


# revision 14
# speedup vs baseline: 1.0222x; 1.0065x over previous
"""FP6Linear (fake-quant-dequant weight + linear) on 8 Trainium2 NeuronCores.

Strategy: column-parallel tensor parallelism. Each core gets a 2048-row shard
of W (out_features) and bias, with x replicated. x is pre-cast to bf16 and
pre-tiled on host into [MT, 128, KB*128] so each m-tile's stationary operand
loads as one fully contiguous 1 MiB DMA.

The FP6 fake-quant-dequant runs on device. The per-tensor scale needs the
abs-max over ALL of W; the sharding replicates one row of W — the row holding
the global abs-max — to every core, so max(|w_extra|) is exactly the global
abs-max with no cross-core traffic (a collective was measured to trip the
board-level GPIO power throttle) and no pass over the shard.

Dequant is exact vs the jax reference but clip-free: scale = amax/16 means
|W*inv| <= 16(1+ulp), and the rne magic-number path maps the +-ulp overhangs
to the same q as the clipped path (16+eps -> q=63, -eps -> q=-0 -> w=c).
Chain: t = W*inv + 16 -> y = t*63/32 + 2^23 (rne) -> q = y - 2^23 (exact
bf16 ints) -> w = q*a + c, balanced across Vector and Scalar (GpSimd
tensor_scalar measured 25x slower).

The matmul runs as two passes over x, one per N-half (1024 columns). In
pass 1 the dequant of half 0 streams concurrently: PSUM groups are
[128, 1024] (2 banks), so 4 m-tiles are in flight and the PE consumes each
arriving half-k-block at 8 matmuls (~1.73 us) while a half-width dequant
chunk takes ~1.5 us to produce — after a ~13 us ramp the PE never starves.
Half 1 dequantizes in the background of pass 1; pass 2 then runs at full
rate. Steady-state issue is ~216 ns per N=512 matmul (~99% of the bf16
streaming peak). DMA rings are split (sync: W/wx/y; scalar: bias/x;
gpsimd: only the partition all-reduce — its SWDGE descriptor-gen is far
too slow for bulk tiles) so the latency-critical weight path never
queues behind bulk traffic.
"""

import numpy as np
import ml_dtypes

import concourse.bacc as bacc
import concourse.bass as bass
import concourse.bass_isa as bass_isa
import concourse.mybir as mybir
import concourse.tile as tile
from concourse import bass_utils

# Problem shapes (hardcoded per contract)
B, S, D_IN, D_OUT = 4, 2048, 4096, 16384
M = B * S               # 8192 rows of x
K = D_IN                # 4096 contraction
N_CORES = 8
N = D_OUT // N_CORES    # 2048 out-features per core
P = 128
KB = K // P             # 32 k-blocks
MT = M // P             # 64 m-tiles
NH = N // 2             # 1024 out-features per pass
NQS = 512               # matmul free dim (one PSUM bank)
WIN = 4                 # m-tiles interleaved during the dequant window

FP32 = mybir.dt.float32
BF16 = mybir.dt.bfloat16
BF16_NP = ml_dtypes.bfloat16
MAGIC = 8388608.0       # 2^23: fp32 add rounds the sum to integer (rne)

_COMPILED = {}


def _build():
    nc = bacc.Bacc(
        "TRN2",
        target_bir_lowering=False,
        debug=False,
        enable_asserts=False,
        num_devices=N_CORES,
    )
    xt_d = nc.dram_tensor("xt", [MT, P, KB * P], BF16, kind="ExternalInput").ap()
    wT_d = nc.dram_tensor("wT", [K, N], FP32, kind="ExternalInput").ap()
    wx_d = nc.dram_tensor("wx", [1, K], FP32, kind="ExternalInput").ap()
    bias_d = nc.dram_tensor("bias", [1, N], FP32, kind="ExternalInput").ap()
    y_d = nc.dram_tensor("y", [M, N], FP32, kind="ExternalOutput").ap()

    with tile.TileContext(nc) as tc:
        with (
            tc.tile_pool(name="const", bufs=1) as const,
            tc.tile_pool(name="wt", bufs=1) as wt_pool,
            tc.tile_pool(name="wl", bufs=3) as wl_pool,
            tc.tile_pool(name="tmp", bufs=2) as tmp_pool,
            tc.tile_pool(name="xt", bufs=5) as xt_pool,
            tc.tile_pool(name="ot", bufs=2) as ot_pool,
            tc.tile_pool(name="psum", bufs=4, space="PSUM") as psum,
        ):
            # ---- global abs-max from the replicated argmax row alone ----
            # (first on the sync DMA ring: the scale chain gates everything)
            wx_sb = const.tile([P, K // P], FP32)
            nc.sync.dma_start(wx_sb[:], wx_d.rearrange("a (p b) -> p (a b)", p=P))
            wx_red = const.tile([P, 1], FP32)
            nc.vector.tensor_reduce(
                wx_red[:], wx_sb[:], mybir.AxisListType.X,
                mybir.AluOpType.max, apply_absolute_value=True,
            )
            g_amax = const.tile([P, 1], FP32)
            nc.gpsimd.partition_all_reduce(
                g_amax[:], wx_red[:], channels=P, reduce_op=bass_isa.ReduceOp.max
            )

            # ---- scale = where(amax > 0, amax/16, 1); inv = 1/scale ----
            m_t = const.tile([P, 1], FP32)
            nc.vector.tensor_scalar(m_t[:], g_amax[:], 0.0, None, mybir.AluOpType.is_gt)
            su = const.tile([P, 1], FP32)
            nc.vector.tensor_scalar(
                su[:], g_amax[:], 1.0 / 16.0, -1.0,
                mybir.AluOpType.mult, mybir.AluOpType.add,
            )
            nc.vector.tensor_tensor(su[:], su[:], m_t[:], mybir.AluOpType.mult)
            scale_t = const.tile([P, 1], FP32)
            nc.vector.tensor_scalar(scale_t[:], su[:], 1.0, None, mybir.AluOpType.add)
            inv_t = const.tile([P, 1], FP32)
            nc.vector.reciprocal(inv_t[:], scale_t[:])
            a_t = const.tile([P, 1], FP32)
            nc.vector.tensor_scalar(a_t[:], scale_t[:], 32.0 / 63.0, None, mybir.AluOpType.mult)
            c_t = const.tile([P, 1], FP32)
            nc.vector.tensor_scalar(c_t[:], scale_t[:], -16.0, None, mybir.AluOpType.mult)

            # bias on the scalar DMA ring (off the W path)
            bias_rep = const.tile([P, N], FP32)
            nc.scalar.dma_start(bias_rep[:], bias_d.to_broadcast((P, N)))

            # ---- dequantize into bf16 W.T SBUF cache, in (half, kb) chunks ----
            # t = W*inv + 16; y = t*63/32 + 2^23 (rne); q = y - 2^23;
            # w = q*a + c   with a = 32/63*scale, c = -16*scale
            wt_sb = wt_pool.tile([P, KB, N], BF16)

            def dequant(kb, lo):
                hi = lo + NH
                wl = wl_pool.tile([P, NH], FP32, tag="wl")
                nc.sync.dma_start(wl[:], wT_d[kb * P : (kb + 1) * P, lo:hi])
                t = tmp_pool.tile([P, NH], FP32, tag="t")
                nc.vector.tensor_scalar(
                    t[:], wl[:], inv_t[:], 16.0,
                    mybir.AluOpType.mult, mybir.AluOpType.add,
                )
                nc.scalar.activation(
                    t[:], t[:], mybir.ActivationFunctionType.Copy,
                    scale=63.0 / 32.0, bias=MAGIC,
                )
                nc.vector.tensor_scalar(
                    wt_sb[:, kb, lo:hi], t[:], -MAGIC, None, mybir.AluOpType.add
                )
                # final affine split across vector + scalar to balance engines
                sp = lo + 512
                nc.vector.tensor_scalar(
                    wt_sb[:, kb, lo:sp], wt_sb[:, kb, lo:sp], a_t[:], c_t[:],
                    mybir.AluOpType.mult, mybir.AluOpType.add,
                )
                nc.scalar.activation(
                    wt_sb[:, kb, sp:hi], wt_sb[:, kb, sp:hi],
                    mybir.ActivationFunctionType.Identity,
                    scale=a_t[:], bias=c_t[:],
                )

            def load_xt(mi):
                xt_t = xt_pool.tile([P, KB * P], BF16, tag="xt")
                nc.scalar.dma_start(xt_t[:], xt_d[mi])
                return xt_t

            def mm(ps, xt_t, kb, q, lo):
                nc.tensor.matmul(
                    ps[:, q * NQS : (q + 1) * NQS],
                    xt_t[:, kb * P : (kb + 1) * P],
                    wt_sb[:, kb, lo + q * NQS : lo + (q + 1) * NQS],
                    start=(kb == 0),
                    stop=(kb == KB - 1),
                )

            def evict(mi, ps, lo):
                ot = ot_pool.tile([P, NH], FP32, tag="ot")
                nc.vector.tensor_tensor(
                    ot[:], ps[:], bias_rep[:, lo : lo + NH], mybir.AluOpType.add
                )
                nc.sync.dma_start(y_d[mi * P : (mi + 1) * P, lo : lo + NH], ot[:])

            # ---- pass 1 (N-half 0): dequant streams concurrently ----
            for kb in range(KB):
                dequant(kb, 0)

            # window: WIN m-tiles interleaved kb-major so the PE consumes
            # each arriving half-k-block at the PSUM-capacity limit
            xts = [load_xt(i) for i in range(WIN)]
            pss = [
                psum.tile([P, NH], FP32, tag="ps", name=f"psw{i}")
                for i in range(WIN)
            ]
            # HAM warm-up: dummy matmuls on xt0 during the otherwise-idle
            # ramp (the real kb=0 groups start with start=True, resetting
            # these banks), so the first real matmuls run at 2.4 GHz
            for i in range(16):
                nc.tensor.matmul(
                    pss[i % WIN][:, 0:NQS],
                    xts[0][:, 0:P],
                    xts[0][:, 0:NQS],
                    start=True,
                    stop=True,
                )
            for kb in range(KB):
                for i in range(WIN):
                    mm(pss[i], xts[i], kb, 0, 0)
                    mm(pss[i], xts[i], kb, 1, 0)
            for i in range(WIN):
                evict(i, pss[i], 0)

            for mi in range(WIN, MT):
                xt_t = load_xt(mi)
                ps = psum.tile([P, NH], FP32, tag="ps")
                for kb in range(KB):
                    mm(ps, xt_t, kb, 0, 0)
                    mm(ps, xt_t, kb, 1, 0)
                # half-1 dequant chunks interleaved into pass 1's engine
                # queues so they overlap the matmul stream (emitted before
                # the eviction so y-writes never block W loads on the ring)
                if WIN <= mi < WIN + KB:
                    dequant(mi - WIN, NH)
                evict(mi, ps, 0)

            # ---- pass 2 (N-half 1): all weights resident, full rate ----
            for mi in range(MT):
                xt_t = load_xt(mi)
                ps = psum.tile([P, NH], FP32, tag="ps")
                for kb in range(KB):
                    mm(ps, xt_t, kb, 0, NH)
                    mm(ps, xt_t, kb, 1, NH)
                evict(mi, ps, NH)

    nc.compile()
    return nc


def _get_compiled():
    if "nc" not in _COMPILED:
        _COMPILED["nc"] = _build()
    return _COMPILED["nc"]


def _make_in_maps(x, W, bias):
    xb = x.reshape(M, K).astype(BF16_NP)
    # [mi, pm, kb, pk] -> [mi, pk, kb, pm]: per-m-tile contiguous K-major tiles
    xt = np.ascontiguousarray(
        xb.reshape(MT, P, KB, P).transpose(0, 3, 2, 1)
    ).reshape(MT, P, KB * P)
    W = np.ascontiguousarray(W.astype(np.float32, copy=False))
    # replicate the W row holding the global abs-max so every core can form
    # the exact global max from local data
    gmax_row = int(np.argmax(np.abs(W)) // K)
    wx = np.ascontiguousarray(W[gmax_row : gmax_row + 1, :])
    in_maps = []
    for c in range(N_CORES):
        wT = np.ascontiguousarray(W[c * N : (c + 1) * N, :].T)
        b = np.ascontiguousarray(bias[c * N : (c + 1) * N].astype(np.float32, copy=False)).reshape(1, N)
        in_maps.append({"xt": xt, "wT": wT, "wx": wx, "bias": b})
    return in_maps


def kernel(x: np.ndarray, W: np.ndarray, bias: np.ndarray) -> np.ndarray:
    assert x.shape == (B, S, D_IN) and W.shape == (D_OUT, D_IN) and bias.shape == (D_OUT,)
    nc = _get_compiled()
    in_maps = _make_in_maps(x, W, bias)
    res = bass_utils.run_bass_kernel_spmd(nc, in_maps, core_ids=list(range(N_CORES)))
    y = np.concatenate([res.results[c]["y"] for c in range(N_CORES)], axis=1)
    return y.reshape(B, S, D_OUT)
